# revision 11
# baseline (speedup 1.0000x reference)
"""MoE (GPT MLP, top-2, GShard capacity) kernel for 8 Trainium2 NeuronCores.

Strategy (expert-parallel, matching the sharding hint):
  - Host: fp32 gate (softmax + top-2 + GShard capacity positions), dispatch
    gather.  Routing is O(N*E) int/scalar work - negligible next to the FFN -
    and the capacity scan is inherently sequential, so it runs on host.
  - Device: 8 cores, core e owns expert e.  Each core runs the expert FFN
    y = gelu(disp @ w1 + b1) @ w2 over its cap=2048 dispatched token slots.
    All matmuls in fp32r (exact fp32 numerics at full PE rate).
  - Host: combine (gather + gate-weighted sum) + b2.

Self-contained: hardcodes B=4, S=2048, D=1024, H=4096, E=8, K=2, cap=2048.
"""

import sys

sys.path.insert(0, "/opt/trn_rl_repo")

import numpy as np

B, S, D, H, E = 4, 2048, 1024, 4096, 8
K = 2
N_TOK = B * S            # 8192
CAP = (K * N_TOK) // E   # 2048 (capacity factor 1.0)
EPS = 1e-9
P = 128                  # SBUF partitions

_NC_CACHE = {}


# --------------------------------------------------------------------------
# Host routing (replicates reference.py's gate exactly, in numpy fp32)
# --------------------------------------------------------------------------

def _route(xt, wg):
    """xt: [N, D] fp32, wg: [D, E] fp32 ->
    gidx [N,K] int, gvals [N,K] fp32 (keep-masked), pos [N,K] int, keep [N,K]"""
    logits = xt @ wg                                   # [N, E] fp32
    m = logits.max(axis=-1, keepdims=True)
    ex = np.exp(logits - m)
    scores = ex / ex.sum(axis=-1, keepdims=True)
    order = np.argsort(-scores, axis=1, kind="stable")  # jax top_k tie rule
    gidx = order[:, :K]                                 # [N, K]
    gvals = np.take_along_axis(scores, gidx, axis=1)
    gvals = gvals / np.clip(gvals.sum(-1, keepdims=True), EPS, None)

    n = xt.shape[0]
    offset = np.zeros(E, np.int64)
    pos = np.zeros((n, K), np.int64)
    keep = np.zeros((n, K), bool)
    rows = np.arange(n)
    for kk in range(K):
        ek = gidx[:, kk]
        oh = np.zeros((n, E), np.int64)
        oh[rows, ek] = 1
        loc = np.cumsum(oh, axis=0) - 1 + offset[None, :]
        offset = offset + oh.sum(axis=0)
        p = loc[rows, ek]
        kmask = p < CAP
        pos[:, kk] = np.where(kmask, p, 0)
        keep[:, kk] = kmask
    gvals = (gvals * keep).astype(np.float32)
    return gidx, gvals, pos, keep


# --------------------------------------------------------------------------
# Device kernel builder (one expert FFN per core, SPMD)
# --------------------------------------------------------------------------

def _build_nc(d, h, ntok, debug=False, act="Gelu"):
    """Expert FFN: y[ntok, d] = gelu(x[ntok, d] @ w1[d, h] + b1[h]) @ w2[h, d].

    Device inputs (pre-laid-out on host for contiguous DMA):
      xt  : [d/P, P, ntok]        x^T tiles (partition = D)
      w1t : [h/P, P, d/P, P]      w1 col-chunks: [m][dp][dt][j] (lhsT layout)
      w2c : [h/P, P, d]           w2 row-chunks
      b1t : [P, h/P]              b1 transposed
    Output:
      y   : [ntok/P, P, d]
    """
    from concourse import bacc, mybir, tile

    dt_n = d // P          # D tiles (contraction tiles for matmul1)
    mt_n = h // P          # H tiles
    nbs = min(512, ntok)   # phase-A token block (moving free dim)
    nb_n = ntok // nbs     # phase-A blocks
    tg_n = ntok // 512 if ntok >= 512 else 1   # phase-B token groups of 512
    tgs = ntok // tg_n                          # tokens per group
    tt_n = tgs // P        # 128-token tiles per group
    dh_n = (d + 511) // 512  # output D half-splits (PSUM bank = 512 fp32)

    f32 = mybir.dt.float32
    f32r = mybir.dt.float32r

    nc = bacc.Bacc("TRN2", target_bir_lowering=False, debug=debug,
                   enable_asserts=False, num_devices=1)

    xt_d = nc.dram_tensor("xt", [dt_n, P, ntok], f32r, kind="ExternalInput")
    w1_d = nc.dram_tensor("w1t", [mt_n, P, dt_n, P], f32r, kind="ExternalInput")
    w2_d = nc.dram_tensor("w2c", [mt_n, P, d], f32r, kind="ExternalInput")
    b1_d = nc.dram_tensor("b1t", [P, mt_n], f32, kind="ExternalInput")
    y_d = nc.dram_tensor("y", [ntok // P, P, d], f32, kind="ExternalOutput")

    with tile.TileContext(nc) as tc:
        with (
            tc.tile_pool(name="xpool", bufs=dt_n) as xpool,
            tc.tile_pool(name="cpool", bufs=1) as cpool,
            tc.tile_pool(name="w1pool", bufs=3) as w1pool,
            tc.tile_pool(name="hpool", bufs=2) as hpool,
            tc.tile_pool(name="w2pool", bufs=3) as w2pool,
            tc.tile_pool(name="hbpool", bufs=3) as hbpool,
            tc.tile_pool(name="ypool", bufs=3) as ypool,
            tc.tile_pool(name="psum", bufs=4, space="PSUM") as psum,
            tc.tile_pool(name="hdram", bufs=1, space="DRAM") as hdram,
        ):
            # resident: x^T tiles + b1
            xts = []
            for dti in range(dt_n):
                xt_t = xpool.tile([P, ntok], f32r, tag="xt")
                nc.sync.dma_start(xt_t[:], xt_d[dti])
                xts.append(xt_t)
            b1_t = cpool.tile([P, mt_n], f32)
            nc.sync.dma_start(b1_t[:], b1_d[:])

            hbuf = hdram.tile([mt_n, P, ntok], f32r)

            # ---- phase A: h = gelu(x @ w1 + b1), streamed per H-tile ----
            for m in range(mt_n):
                w1m = w1pool.tile([P, dt_n, P], f32r, tag="w1m")
                nc.sync.dma_start(w1m[:], w1_d[m])
                hm = hpool.tile([P, ntok], f32, tag="hm")
                for n in range(nb_n):
                    ps = psum.tile([P, 512 * dh_n], f32, tag="ps")
                    for dti in range(dt_n):
                        nc.tensor.matmul(
                            ps[:, 0:nbs],
                            w1m[:, dti, :],
                            xts[dti][:, n * nbs:(n + 1) * nbs],
                            start=(dti == 0),
                            stop=(dti == dt_n - 1),
                        )
                    nc.scalar.activation(
                        hm[:, n * nbs:(n + 1) * nbs],
                        ps[:, 0:nbs],
                        getattr(mybir.ActivationFunctionType, act),
                        bias=b1_t[:, m:m + 1],
                        scale=1.0,
                    )
                nc.sync.dma_start(hbuf[m], hm[:].bitcast(f32r))

            # ---- phase B: y = h @ w2, PSUM-accumulated over H ----
            for tg in range(tg_n):
                pss = [psum.tile([P, 512 * dh_n], f32, tag="ps", name=f"pss_{tg}_{t}")
                       for t in range(tt_n)]
                for m in range(mt_n):
                    w2m = w2pool.tile([P, d], f32r, tag="w2m")
                    nc.sync.dma_start(w2m[:], w2_d[m])
                    hbm = hbpool.tile([P, tgs], f32r, tag="hbm")
                    nc.sync.dma_start(hbm[:], hbuf[m][:, tg * tgs:(tg + 1) * tgs])
                    for t in range(tt_n):
                        for dh in range(dh_n):
                            c0, c1 = dh * 512, min((dh + 1) * 512, d)
                            nc.tensor.matmul(
                                pss[t][:, c0:c1],
                                hbm[:, t * P:(t + 1) * P],
                                w2m[:, c0:c1],
                                start=(m == 0),
                                stop=(m == mt_n - 1),
                            )
                for t in range(tt_n):
                    yt = ypool.tile([P, d], f32, tag="yt")
                    nc.vector.tensor_copy(yt[:], pss[t][:, 0:d])
                    nc.sync.dma_start(y_d[tg * tt_n + t], yt[:])

    nc.compile()
    return nc


def _get_nc(d, h, ntok, debug=False):
    key = (d, h, ntok, debug)
    if key not in _NC_CACHE:
        _NC_CACHE[key] = _build_nc(d, h, ntok, debug)
    return _NC_CACHE[key]


# --------------------------------------------------------------------------
# Host-side input layout per core
# --------------------------------------------------------------------------

def _core_inputs(disp_e, w1_e, b1_e):
    """disp_e: [CAP, D], w1_e: [D, H], b1_e: [H] -> device input dict."""
    xt = np.ascontiguousarray(disp_e.T).reshape(D // P, P, CAP)
    w1t = np.ascontiguousarray(
        w1_e.reshape(D // P, P, H // P, P).transpose(2, 1, 0, 3))
    w2c = None  # filled by caller
    b1t = np.ascontiguousarray(b1_e.reshape(H // P, P).T)
    return xt, w1t, b1t


def _get_runner(nc, n_cores):
    """Cached PJRT executable for an SPMD bass program (axon path of
    run_bass_kernel_spmd, with the jitted callable kept warm across calls)."""
    key = id(nc)
    if key in _NC_CACHE:
        return _NC_CACHE[key]

    import jax
    from jax.sharding import Mesh, PartitionSpec
    from jax.experimental.shard_map import shard_map
    from concourse import mybir
    from concourse.bass2jax import (_bass_exec_p, install_neuronx_cc_hook,
                                    partition_id_tensor)

    install_neuronx_cc_hook()

    partition_name = (nc.partition_id_tensor.name
                      if nc.partition_id_tensor else None)
    in_names, out_names, out_avals = [], [], []
    for alloc in nc.m.functions[0].allocations:
        if not isinstance(alloc, mybir.MemoryLocationSet):
            continue
        name = alloc.memorylocations[0].name
        if alloc.kind == "ExternalInput":
            if name != partition_name:
                in_names.append(name)
        elif alloc.kind == "ExternalOutput":
            out_names.append(name)
            shape = tuple(alloc.tensor_shape)
            out_avals.append(jax.core.ShapedArray(shape, mybir.dt.np(alloc.dtype)))
    n_params = len(in_names)
    n_outs = len(out_avals)
    in_names = in_names + out_names
    if partition_name is not None:
        in_names.append(partition_name)
    donate = tuple(range(n_params, n_params + n_outs))

    def _body(*args):
        operands = list(args)
        if partition_name is not None:
            operands.append(partition_id_tensor())
        outs = _bass_exec_p.bind(
            *operands,
            out_avals=tuple(out_avals),
            in_names=tuple(in_names),
            out_names=tuple(out_names),
            lowering_input_output_aliases=(),
            sim_require_finite=True,
            sim_require_nnan=True,
            nc=nc,
        )
        return tuple(outs)

    devices = jax.devices()[:n_cores]
    mesh = Mesh(np.asarray(devices), ("core",))
    in_specs = (PartitionSpec("core"),) * (n_params + n_outs)
    out_specs = (PartitionSpec("core"),) * n_outs
    sharded = jax.jit(
        shard_map(_body, mesh=mesh, in_specs=in_specs, out_specs=out_specs,
                  check_rep=False),
        donate_argnums=donate, keep_unused=True,
    )

    def run(in_maps, reps=1, time_reps=False):
        import time as _time
        concat_in = [
            np.concatenate([np.asarray(m[in_names[i]]) for m in in_maps], axis=0)
            for i in range(n_params)
        ]
        concat_in = [jax.device_put(a) for a in concat_in]
        zero_sets = []
        for _ in range(reps):
            zero_sets.append([
                jax.device_put(np.zeros((n_cores * av.shape[0], *av.shape[1:]),
                                        av.dtype))
                for av in out_avals
            ])
        for zs in zero_sets:
            for z in zs:
                z.block_until_ready()
        for a in concat_in:
            a.block_until_ready()
        times = []
        out_arrs = None
        for r in range(reps):
            t0 = _time.perf_counter()
            out_arrs = sharded(*concat_in, *zero_sets[r])
            for o in out_arrs:
                o.block_until_ready()
            times.append(_time.perf_counter() - t0)
        results = [
            {name: np.asarray(out_arrs[i]).reshape(n_cores, *out_avals[i].shape)[c]
             for i, name in enumerate(out_names)}
            for c in range(n_cores)
        ]
        if time_reps:
            return results, times
        return results

    _NC_CACHE[key] = run
    return run


def kernel(x, wg, w1, b1, w2, b2):

    x = np.asarray(x, np.float32)
    wg = np.asarray(wg, np.float32)
    w1 = np.asarray(w1, np.float32)
    b1 = np.asarray(b1, np.float32)
    w2 = np.asarray(w2, np.float32)
    b2 = np.asarray(b2, np.float32)

    xt = x.reshape(N_TOK, D)
    gidx, gvals, pos, keep = _route(xt, wg)

    # dispatch: slots are unique per expert, so assignment == scatter-add
    disp = np.zeros((E, CAP, D), np.float32)
    for kk in range(K):
        tok = np.nonzero(keep[:, kk])[0]
        disp[gidx[tok, kk], pos[tok, kk]] = xt[tok]

    in_maps = []
    for e in range(E):
        xte, w1te, b1te = _core_inputs(disp[e], w1[e], b1[e])
        in_maps.append({
            "xt": xte,
            "w1t": w1te,
            "w2c": np.ascontiguousarray(w2[e]).reshape(H // P, P, D),
            "b1t": b1te,
        })

    nc = _get_nc(D, H, CAP)
    run = _get_runner(nc, E)
    results = run(in_maps)
    y_all = np.stack([r["y"].reshape(CAP, D) for r in results])  # [E,CAP,D]

    # combine: out = sum_k gvals * (y[e, pos] + b2[e])
    e_flat = gidx.reshape(-1)
    p_flat = pos.reshape(-1)
    yk = y_all[e_flat, p_flat] + b2[e_flat]
    w = gvals.reshape(-1).astype(np.float32)
    out = (yk * w[:, None]).reshape(N_TOK, K, D).sum(axis=1)
    return out.reshape(B, S, D).astype(np.float32)


# --------------------------------------------------------------------------
# Benchmarking helpers (test.py only)
# --------------------------------------------------------------------------

def _build_null_nc():
    """Minimal kernel (one 64KB copy) to calibrate per-call dispatch overhead."""
    from concourse import bacc, mybir, tile

    f32 = mybir.dt.float32
    nc = bacc.Bacc("TRN2", target_bir_lowering=False, debug=False,
                   enable_asserts=False, num_devices=1)
    a_d = nc.dram_tensor("a", [P, P], f32, kind="ExternalInput")
    o_d = nc.dram_tensor("o", [P, P], f32, kind="ExternalOutput")
    with tile.TileContext(nc) as tc:
        with tc.tile_pool(name="pool", bufs=1) as pool:
            t = pool.tile([P, P], f32)
            nc.sync.dma_start(t[:], a_d[:])
            nc.sync.dma_start(o_d[:], t[:])
    nc.compile()
    return nc


def bench(x, wg, w1, b1, w2, b2, reps=10):
    """Returns (real_times, null_times) lists of per-call wall seconds."""
    x = np.asarray(x, np.float32)
    xt = x.reshape(N_TOK, D)
    gidx, gvals, pos, keep = _route(xt, np.asarray(wg, np.float32))
    disp = np.zeros((E, CAP, D), np.float32)
    for kk in range(K):
        tok = np.nonzero(keep[:, kk])[0]
        disp[gidx[tok, kk], pos[tok, kk]] = xt[tok]
    in_maps = []
    for e in range(E):
        xte, w1te, b1te = _core_inputs(disp[e], np.asarray(w1[e], np.float32),
                                       np.asarray(b1[e], np.float32))
        in_maps.append({
            "xt": xte,
            "w1t": w1te,
            "w2c": np.ascontiguousarray(np.asarray(w2[e], np.float32)).reshape(H // P, P, D),
            "b1t": b1te,
        })
    nc = _get_nc(D, H, CAP)
    run = _get_runner(nc, E)
    _, real_times = run(in_maps, reps=reps, time_reps=True)

    if "null" not in _NC_CACHE:
        _NC_CACHE["null"] = _build_null_nc()
    null_nc = _NC_CACHE["null"]
    null_run = _get_runner(null_nc, E)
    null_maps = [{"a": np.zeros((P, P), np.float32)} for _ in range(E)]
    _, null_times = null_run(null_maps, reps=reps, time_reps=True)
    return real_times, null_times


# revision 18
# speedup vs baseline: 33.9933x; 33.9933x over previous
"""MoE (GPT MLP, top-2, GShard capacity) kernel for 8 Trainium2 NeuronCores.

Strategy (expert-parallel, matching the sharding hint):
  - Host: fp32 gate (softmax + top-2 + GShard capacity positions), dispatch
    gather.  Routing is O(N*E) int/scalar work - negligible next to the FFN -
    and the capacity scan is inherently sequential, so it runs on host.
  - Device: 8 cores, core e owns expert e.  Each core runs the expert FFN
    y = gelu(disp @ w1 + b1) @ w2 over its cap=2048 dispatched token slots.
    All matmuls in fp32r (exact fp32 numerics at full PE rate).
  - Host: combine (gather + gate-weighted sum) + b2.

Self-contained: hardcodes B=4, S=2048, D=1024, H=4096, E=8, K=2, cap=2048.
"""

import sys

sys.path.insert(0, "/opt/trn_rl_repo")

import numpy as np

B, S, D, H, E = 4, 2048, 1024, 4096, 8
K = 2
N_TOK = B * S            # 8192
CAP = (K * N_TOK) // E   # 2048 (capacity factor 1.0)
EPS = 1e-9
P = 128                  # SBUF partitions

_NC_CACHE = {}


# --------------------------------------------------------------------------
# Host routing (replicates reference.py's gate exactly, in numpy fp32)
# --------------------------------------------------------------------------

def _route(xt, wg):
    """xt: [N, D] fp32, wg: [D, E] fp32 ->
    gidx [N,K] int, gvals [N,K] fp32 (keep-masked), pos [N,K] int, keep [N,K]"""
    logits = xt @ wg                                   # [N, E] fp32
    m = logits.max(axis=-1, keepdims=True)
    ex = np.exp(logits - m)
    scores = ex / ex.sum(axis=-1, keepdims=True)
    order = np.argsort(-scores, axis=1, kind="stable")  # jax top_k tie rule
    gidx = order[:, :K]                                 # [N, K]
    gvals = np.take_along_axis(scores, gidx, axis=1)
    gvals = gvals / np.clip(gvals.sum(-1, keepdims=True), EPS, None)

    n = xt.shape[0]
    offset = np.zeros(E, np.int64)
    pos = np.zeros((n, K), np.int64)
    keep = np.zeros((n, K), bool)
    rows = np.arange(n)
    for kk in range(K):
        ek = gidx[:, kk]
        oh = np.zeros((n, E), np.int64)
        oh[rows, ek] = 1
        loc = np.cumsum(oh, axis=0) - 1 + offset[None, :]
        offset = offset + oh.sum(axis=0)
        p = loc[rows, ek]
        kmask = p < CAP
        pos[:, kk] = np.where(kmask, p, 0)
        keep[:, kk] = kmask
    gvals = (gvals * keep).astype(np.float32)
    return gidx, gvals, pos, keep


# --------------------------------------------------------------------------
# Device kernel builder (one expert FFN per core, SPMD)
# --------------------------------------------------------------------------

def _build_nc(d, h, ntok, debug=False, act="Gelu", reps=1):
    """Expert FFN: y[ntok, d] = gelu(x[ntok, d] @ w1[d, h] + b1[h]) @ w2[h, d].

    Device inputs (pre-laid-out on host for contiguous DMA):
      xt  : [d/P, P, ntok]        x^T tiles (partition = D)
      w1t : [h/P, P, d/P, P]      w1 col-chunks: [m][dp][dt][j] (lhsT layout)
      w2c : [h/P, P, d]           w2 row-chunks
      b1t : [P, h/P]              b1 transposed
    Output:
      y   : [ntok/P, P, d]
    """
    from concourse import bacc, mybir, tile

    dt_n = d // P          # D tiles (contraction tiles for matmul1)
    mt_n = h // P          # H tiles
    nbs = min(512, ntok)   # phase-A token block (moving free dim)
    nb_n = ntok // nbs     # phase-A blocks
    tg_n = ntok // 512 if ntok >= 512 else 1   # phase-B token groups of 512
    tgs = ntok // tg_n                          # tokens per group
    tt_n = tgs // P        # 128-token tiles per group
    dh_n = (d + 511) // 512  # output D half-splits (PSUM bank = 512 fp32)

    f32 = mybir.dt.float32
    f32r = mybir.dt.float32r

    nc = bacc.Bacc("TRN2", target_bir_lowering=False, debug=debug,
                   enable_asserts=False, num_devices=1)

    xt_d = nc.dram_tensor("xt", [dt_n, P, ntok], f32r, kind="ExternalInput")
    w1_d = nc.dram_tensor("w1t", [mt_n, P, dt_n, P], f32r, kind="ExternalInput")
    w2_d = nc.dram_tensor("w2c", [mt_n, P, d], f32r, kind="ExternalInput")
    b1_d = nc.dram_tensor("b1t", [P, mt_n], f32, kind="ExternalInput")
    y_d = nc.dram_tensor("y", [ntok // P, P, d], f32, kind="ExternalOutput")

    with tile.TileContext(nc) as tc:
        with (
            tc.tile_pool(name="xpool", bufs=dt_n) as xpool,
            tc.tile_pool(name="cpool", bufs=1) as cpool,
            tc.tile_pool(name="w1pool", bufs=3) as w1pool,
            tc.tile_pool(name="hpool", bufs=2) as hpool,
            tc.tile_pool(name="w2pool", bufs=3) as w2pool,
            tc.tile_pool(name="hbpool", bufs=3) as hbpool,
            tc.tile_pool(name="ypool", bufs=3) as ypool,
            tc.tile_pool(name="psum", bufs=4, space="PSUM") as psum,
            tc.tile_pool(name="hdram", bufs=1, space="DRAM") as hdram,
        ):
            # resident: x^T tiles + b1
            xts = []
            for dti in range(dt_n):
                xt_t = xpool.tile([P, ntok], f32r, tag="xt")
                nc.sync.dma_start(xt_t[:], xt_d[dti])
                xts.append(xt_t)
            b1_t = cpool.tile([P, mt_n], f32)
            nc.sync.dma_start(b1_t[:], b1_d[:])

            hbuf = hdram.tile([mt_n, P, ntok], f32r)

            for _rep in range(reps):
                self_body(nc, tc, mybir, act, f32, f32r,
                          dt_n, mt_n, nbs, nb_n, tg_n, tgs, tt_n, dh_n, d, ntok,
                          xts, b1_t, hbuf, w1_d, w2_d, y_d,
                          w1pool, hpool, w2pool, hbpool, ypool, psum, _rep)

    nc.compile()
    return nc


def self_body(nc, tc, mybir, act, f32, f32r,
              dt_n, mt_n, nbs, nb_n, tg_n, tgs, tt_n, dh_n, d, ntok,
              xts, b1_t, hbuf, w1_d, w2_d, y_d,
              w1pool, hpool, w2pool, hbpool, ypool, psum, rep):
    if True:
        if True:
            # ---- phase A: h = gelu(x @ w1 + b1), streamed per H-tile ----
            for m in range(mt_n):
                w1m = w1pool.tile([P, dt_n, P], f32r, tag="w1m")
                nc.sync.dma_start(w1m[:], w1_d[m])
                hm = hpool.tile([P, ntok], f32, tag="hm")
                for n in range(nb_n):
                    ps = psum.tile([P, 512 * dh_n], f32, tag="ps")
                    for dti in range(dt_n):
                        nc.tensor.matmul(
                            ps[:, 0:nbs],
                            w1m[:, dti, :],
                            xts[dti][:, n * nbs:(n + 1) * nbs],
                            start=(dti == 0),
                            stop=(dti == dt_n - 1),
                        )
                    nc.scalar.activation(
                        hm[:, n * nbs:(n + 1) * nbs],
                        ps[:, 0:nbs],
                        getattr(mybir.ActivationFunctionType, act),
                        bias=b1_t[:, m:m + 1],
                        scale=1.0,
                    )
                nc.sync.dma_start(hbuf[m], hm[:].bitcast(f32r))

            # ---- phase B: y = h @ w2, PSUM-accumulated over H ----
            for tg in range(tg_n):
                pss = [psum.tile([P, 512 * dh_n], f32, tag="ps",
                                 name=f"pss_{rep}_{tg}_{t}")
                       for t in range(tt_n)]
                for m in range(mt_n):
                    w2m = w2pool.tile([P, d], f32r, tag="w2m")
                    nc.sync.dma_start(w2m[:], w2_d[m])
                    hbm = hbpool.tile([P, tgs], f32r, tag="hbm")
                    nc.sync.dma_start(hbm[:], hbuf[m][:, tg * tgs:(tg + 1) * tgs])
                    for t in range(tt_n):
                        for dh in range(dh_n):
                            c0, c1 = dh * 512, min((dh + 1) * 512, d)
                            nc.tensor.matmul(
                                pss[t][:, c0:c1],
                                hbm[:, t * P:(t + 1) * P],
                                w2m[:, c0:c1],
                                start=(m == 0),
                                stop=(m == mt_n - 1),
                            )
                for t in range(tt_n):
                    yt = ypool.tile([P, d], f32, tag="yt")
                    nc.vector.tensor_copy(yt[:], pss[t][:, 0:d])
                    nc.sync.dma_start(y_d[tg * tt_n + t], yt[:])


def _get_nc(d, h, ntok, debug=False, reps=1):
    key = (d, h, ntok, debug, reps)
    if key not in _NC_CACHE:
        _NC_CACHE[key] = _build_nc(d, h, ntok, debug, reps=reps)
    return _NC_CACHE[key]


# --------------------------------------------------------------------------
# Host-side input layout per core
# --------------------------------------------------------------------------

def _core_inputs(disp_e, w1_e, b1_e):
    """disp_e: [CAP, D], w1_e: [D, H], b1_e: [H] -> device input dict."""
    xt = np.ascontiguousarray(disp_e.T).reshape(D // P, P, CAP)
    w1t = np.ascontiguousarray(
        w1_e.reshape(D // P, P, H // P, P).transpose(2, 1, 0, 3))
    w2c = None  # filled by caller
    b1t = np.ascontiguousarray(b1_e.reshape(H // P, P).T)
    return xt, w1t, b1t


def _get_runner(nc, n_cores):
    """Cached PJRT executable for an SPMD bass program (axon path of
    run_bass_kernel_spmd, with the jitted callable kept warm across calls)."""
    key = id(nc)
    if key in _NC_CACHE:
        return _NC_CACHE[key]

    import jax
    from jax.sharding import Mesh, PartitionSpec
    from jax.experimental.shard_map import shard_map
    from concourse import mybir
    from concourse.bass2jax import (_bass_exec_p, install_neuronx_cc_hook,
                                    partition_id_tensor)

    install_neuronx_cc_hook()

    partition_name = (nc.partition_id_tensor.name
                      if nc.partition_id_tensor else None)
    in_names, out_names, out_avals = [], [], []
    for alloc in nc.m.functions[0].allocations:
        if not isinstance(alloc, mybir.MemoryLocationSet):
            continue
        name = alloc.memorylocations[0].name
        if alloc.kind == "ExternalInput":
            if name != partition_name:
                in_names.append(name)
        elif alloc.kind == "ExternalOutput":
            out_names.append(name)
            shape = tuple(alloc.tensor_shape)
            out_avals.append(jax.core.ShapedArray(shape, mybir.dt.np(alloc.dtype)))
    n_params = len(in_names)
    n_outs = len(out_avals)
    in_names = in_names + out_names
    if partition_name is not None:
        in_names.append(partition_name)
    donate = tuple(range(n_params, n_params + n_outs))

    def _body(*args):
        operands = list(args)
        if partition_name is not None:
            operands.append(partition_id_tensor())
        outs = _bass_exec_p.bind(
            *operands,
            out_avals=tuple(out_avals),
            in_names=tuple(in_names),
            out_names=tuple(out_names),
            lowering_input_output_aliases=(),
            sim_require_finite=True,
            sim_require_nnan=True,
            nc=nc,
        )
        return tuple(outs)

    devices = jax.devices()[:n_cores]
    mesh = Mesh(np.asarray(devices), ("core",))
    in_specs = (PartitionSpec("core"),) * (n_params + n_outs)
    out_specs = (PartitionSpec("core"),) * n_outs
    sharded = jax.jit(
        shard_map(_body, mesh=mesh, in_specs=in_specs, out_specs=out_specs,
                  check_rep=False),
        donate_argnums=donate, keep_unused=True,
    )

    def run(in_maps, reps=1, time_reps=False):
        import time as _time
        concat_in = [
            np.concatenate([np.asarray(m[in_names[i]]) for m in in_maps], axis=0)
            for i in range(n_params)
        ]
        concat_in = [jax.device_put(a) for a in concat_in]
        zero_sets = []
        for _ in range(reps):
            zero_sets.append([
                jax.device_put(np.zeros((n_cores * av.shape[0], *av.shape[1:]),
                                        av.dtype))
                for av in out_avals
            ])
        for zs in zero_sets:
            for z in zs:
                z.block_until_ready()
        for a in concat_in:
            a.block_until_ready()
        times = []
        out_arrs = None
        for r in range(reps):
            t0 = _time.perf_counter()
            out_arrs = sharded(*concat_in, *zero_sets[r])
            for o in out_arrs:
                o.block_until_ready()
            times.append(_time.perf_counter() - t0)
        results = [
            {name: np.asarray(out_arrs[i]).reshape(n_cores, *out_avals[i].shape)[c]
             for i, name in enumerate(out_names)}
            for c in range(n_cores)
        ]
        if time_reps:
            return results, times
        return results

    _NC_CACHE[key] = run
    return run


def kernel(x, wg, w1, b1, w2, b2):

    x = np.asarray(x, np.float32)
    wg = np.asarray(wg, np.float32)
    w1 = np.asarray(w1, np.float32)
    b1 = np.asarray(b1, np.float32)
    w2 = np.asarray(w2, np.float32)
    b2 = np.asarray(b2, np.float32)

    xt = x.reshape(N_TOK, D)
    gidx, gvals, pos, keep = _route(xt, wg)

    # dispatch: slots are unique per expert, so assignment == scatter-add
    disp = np.zeros((E, CAP, D), np.float32)
    for kk in range(K):
        tok = np.nonzero(keep[:, kk])[0]
        disp[gidx[tok, kk], pos[tok, kk]] = xt[tok]

    in_maps = []
    for e in range(E):
        xte, w1te, b1te = _core_inputs(disp[e], w1[e], b1[e])
        in_maps.append({
            "xt": xte,
            "w1t": w1te,
            "w2c": np.ascontiguousarray(w2[e]).reshape(H // P, P, D),
            "b1t": b1te,
        })

    nc = _get_nc(D, H, CAP)
    run = _get_runner(nc, E)
    results = run(in_maps)
    y_all = np.stack([r["y"].reshape(CAP, D) for r in results])  # [E,CAP,D]

    # combine: out = sum_k gvals * (y[e, pos] + b2[e])
    e_flat = gidx.reshape(-1)
    p_flat = pos.reshape(-1)
    yk = y_all[e_flat, p_flat] + b2[e_flat]
    w = gvals.reshape(-1).astype(np.float32)
    out = (yk * w[:, None]).reshape(N_TOK, K, D).sum(axis=1)
    return out.reshape(B, S, D).astype(np.float32)


# --------------------------------------------------------------------------
# Benchmarking helpers (test.py only)
# --------------------------------------------------------------------------

def _build_null_nc():
    """Minimal kernel (one 64KB copy) to calibrate per-call dispatch overhead."""
    from concourse import bacc, mybir, tile

    f32 = mybir.dt.float32
    nc = bacc.Bacc("TRN2", target_bir_lowering=False, debug=False,
                   enable_asserts=False, num_devices=1)
    a_d = nc.dram_tensor("a", [P, P], f32, kind="ExternalInput")
    o_d = nc.dram_tensor("o", [P, P], f32, kind="ExternalOutput")
    with tile.TileContext(nc) as tc:
        with tc.tile_pool(name="pool", bufs=1) as pool:
            t = pool.tile([P, P], f32)
            nc.sync.dma_start(t[:], a_d[:])
            nc.sync.dma_start(o_d[:], t[:])
    nc.compile()
    return nc


def bench(x, wg, w1, b1, w2, b2, reps=10):
    """Returns (real_times, null_times) lists of per-call wall seconds."""
    x = np.asarray(x, np.float32)
    xt = x.reshape(N_TOK, D)
    gidx, gvals, pos, keep = _route(xt, np.asarray(wg, np.float32))
    disp = np.zeros((E, CAP, D), np.float32)
    for kk in range(K):
        tok = np.nonzero(keep[:, kk])[0]
        disp[gidx[tok, kk], pos[tok, kk]] = xt[tok]
    in_maps = []
    for e in range(E):
        xte, w1te, b1te = _core_inputs(disp[e], np.asarray(w1[e], np.float32),
                                       np.asarray(b1[e], np.float32))
        in_maps.append({
            "xt": xte,
            "w1t": w1te,
            "w2c": np.ascontiguousarray(np.asarray(w2[e], np.float32)).reshape(H // P, P, D),
            "b1t": b1te,
        })
    nc1 = _get_nc(D, H, CAP, reps=1)
    run1 = _get_runner(nc1, E)
    _, t1 = run1(in_maps, reps=reps, time_reps=True)

    nc3 = _get_nc(D, H, CAP, reps=3)
    run3 = _get_runner(nc3, E)
    _, t3 = run3(in_maps, reps=reps, time_reps=True)
    return t1, t3


# revision 19
# speedup vs baseline: 59.5327x; 1.7513x over previous
"""MoE (GPT MLP, top-2, GShard capacity) kernel for 8 Trainium2 NeuronCores.

Strategy (expert-parallel, matching the sharding hint):
  - Host: fp32 gate (softmax + top-2 + GShard capacity positions), dispatch
    gather.  Routing is O(N*E) int/scalar work - negligible next to the FFN -
    and the capacity scan is inherently sequential, so it runs on host.
  - Device: 8 cores, core e owns expert e.  Each core runs the expert FFN
    y = gelu(disp @ w1 + b1) @ w2 over its cap=2048 dispatched token slots.
    All matmuls in fp32r (exact fp32 numerics at full PE rate).
  - Host: combine (gather + gate-weighted sum) + b2.

Self-contained: hardcodes B=4, S=2048, D=1024, H=4096, E=8, K=2, cap=2048.
"""

import sys

sys.path.insert(0, "/opt/trn_rl_repo")

import numpy as np

B, S, D, H, E = 4, 2048, 1024, 4096, 8
K = 2
N_TOK = B * S            # 8192
CAP = (K * N_TOK) // E   # 2048 (capacity factor 1.0)
EPS = 1e-9
P = 128                  # SBUF partitions

_NC_CACHE = {}


# --------------------------------------------------------------------------
# Host routing (replicates reference.py's gate exactly, in numpy fp32)
# --------------------------------------------------------------------------

def _route(xt, wg):
    """xt: [N, D] fp32, wg: [D, E] fp32 ->
    gidx [N,K] int, gvals [N,K] fp32 (keep-masked), pos [N,K] int, keep [N,K]"""
    logits = xt @ wg                                   # [N, E] fp32
    m = logits.max(axis=-1, keepdims=True)
    ex = np.exp(logits - m)
    scores = ex / ex.sum(axis=-1, keepdims=True)
    order = np.argsort(-scores, axis=1, kind="stable")  # jax top_k tie rule
    gidx = order[:, :K]                                 # [N, K]
    gvals = np.take_along_axis(scores, gidx, axis=1)
    gvals = gvals / np.clip(gvals.sum(-1, keepdims=True), EPS, None)

    n = xt.shape[0]
    offset = np.zeros(E, np.int64)
    pos = np.zeros((n, K), np.int64)
    keep = np.zeros((n, K), bool)
    rows = np.arange(n)
    for kk in range(K):
        ek = gidx[:, kk]
        oh = np.zeros((n, E), np.int64)
        oh[rows, ek] = 1
        loc = np.cumsum(oh, axis=0) - 1 + offset[None, :]
        offset = offset + oh.sum(axis=0)
        p = loc[rows, ek]
        kmask = p < CAP
        pos[:, kk] = np.where(kmask, p, 0)
        keep[:, kk] = kmask
    gvals = (gvals * keep).astype(np.float32)
    return gidx, gvals, pos, keep


# --------------------------------------------------------------------------
# Device kernel builder (one expert FFN per core, SPMD)
# --------------------------------------------------------------------------

def _build_nc(d, h, ntok, debug=False, act="Gelu", reps=1):
    """Expert FFN: y[ntok, d] = gelu(x[ntok, d] @ w1[d, h] + b1[h]) @ w2[h, d].

    Device inputs (pre-laid-out on host for contiguous DMA):
      xt  : [d/P, P, ntok]        x^T tiles (partition = D)
      w1t : [h/P, P, d/P, P]      w1 col-chunks: [m][dp][dt][j] (lhsT layout)
      w2c : [h/P, P, d]           w2 row-chunks
      b1t : [P, h/P]              b1 transposed
    Output:
      y   : [ntok/P, P, d]
    """
    from concourse import bacc, mybir, tile

    dt_n = d // P          # D tiles (contraction tiles for matmul1)
    mt_n = h // P          # H tiles
    nbs = min(512, ntok)   # phase-A token block (moving free dim)
    nb_n = ntok // nbs     # phase-A blocks
    tg_n = ntok // 512 if ntok >= 512 else 1   # phase-B token groups of 512
    tgs = ntok // tg_n                          # tokens per group
    tt_n = tgs // P        # 128-token tiles per group
    dh_n = (d + 511) // 512  # output D half-splits (PSUM bank = 512 fp32)

    f32 = mybir.dt.float32
    f32r = mybir.dt.float32r

    nc = bacc.Bacc("TRN2", target_bir_lowering=False, debug=debug,
                   enable_asserts=False, num_devices=1)

    xt_d = nc.dram_tensor("xt", [dt_n, P, ntok], f32r, kind="ExternalInput")
    w1_d = nc.dram_tensor("w1t", [mt_n, P, dt_n, P], f32r, kind="ExternalInput")
    w2_d = nc.dram_tensor("w2c", [mt_n, P, d], f32r, kind="ExternalInput")
    b1_d = nc.dram_tensor("b1t", [P, mt_n], f32, kind="ExternalInput")
    y_d = nc.dram_tensor("y", [ntok // P, P, d], f32, kind="ExternalOutput")

    with tile.TileContext(nc) as tc:
        with (
            tc.tile_pool(name="xpool", bufs=dt_n) as xpool,
            tc.tile_pool(name="cpool", bufs=1) as cpool,
            tc.tile_pool(name="w1pool", bufs=6) as w1pool,
            tc.tile_pool(name="hpool", bufs=3) as hpool,
            tc.tile_pool(name="w2pool", bufs=8) as w2pool,
            tc.tile_pool(name="hbpool", bufs=8) as hbpool,
            tc.tile_pool(name="ypool", bufs=4) as ypool,
            tc.tile_pool(name="psum", bufs=4, space="PSUM") as psum,
            tc.tile_pool(name="hdram", bufs=1, space="DRAM") as hdram,
        ):
            # resident: x^T tiles + b1
            xts = []
            for dti in range(dt_n):
                xt_t = xpool.tile([P, ntok], f32r, tag="xt")
                nc.sync.dma_start(xt_t[:], xt_d[dti])
                xts.append(xt_t)
            b1_t = cpool.tile([P, mt_n], f32)
            nc.sync.dma_start(b1_t[:], b1_d[:])

            hbuf = hdram.tile([mt_n, P, ntok], f32r)

            for _rep in range(reps):
                self_body(nc, tc, mybir, act, f32, f32r,
                          dt_n, mt_n, nbs, nb_n, tg_n, tgs, tt_n, dh_n, d, ntok,
                          xts, b1_t, hbuf, w1_d, w2_d, y_d,
                          w1pool, hpool, w2pool, hbpool, ypool, psum, _rep)

    nc.compile()
    return nc


def self_body(nc, tc, mybir, act, f32, f32r,
              dt_n, mt_n, nbs, nb_n, tg_n, tgs, tt_n, dh_n, d, ntok,
              xts, b1_t, hbuf, w1_d, w2_d, y_d,
              w1pool, hpool, w2pool, hbpool, ypool, psum, rep):
    if True:
        if True:
            # ---- phase A: h = gelu(x @ w1 + b1), streamed per H-tile ----
            for m in range(mt_n):
                w1m = w1pool.tile([P, dt_n, P], f32r, tag="w1m")
                nc.sync.dma_start(w1m[:], w1_d[m])
                hm = hpool.tile([P, ntok], f32, tag="hm")
                for n in range(nb_n):
                    ps = psum.tile([P, 512 * dh_n], f32, tag="ps")
                    for dti in range(dt_n):
                        nc.tensor.matmul(
                            ps[:, 0:nbs],
                            w1m[:, dti, :],
                            xts[dti][:, n * nbs:(n + 1) * nbs],
                            start=(dti == 0),
                            stop=(dti == dt_n - 1),
                        )
                    nc.scalar.activation(
                        hm[:, n * nbs:(n + 1) * nbs],
                        ps[:, 0:nbs],
                        getattr(mybir.ActivationFunctionType, act),
                        bias=b1_t[:, m:m + 1],
                        scale=1.0,
                    )
                nc.sync.dma_start(hbuf[m], hm[:].bitcast(f32r))

            # ---- phase B: y = h @ w2, PSUM-accumulated over H ----
            for tg in range(tg_n):
                pss = [psum.tile([P, 512 * dh_n], f32, tag="ps",
                                 name=f"pss_{rep}_{tg}_{t}")
                       for t in range(tt_n)]
                for m in range(mt_n):
                    w2m = w2pool.tile([P, d], f32r, tag="w2m")
                    nc.sync.dma_start(w2m[:], w2_d[m])
                    hbm = hbpool.tile([P, tgs], f32r, tag="hbm")
                    nc.sync.dma_start(hbm[:], hbuf[m][:, tg * tgs:(tg + 1) * tgs])
                    for t in range(tt_n):
                        for dh in range(dh_n):
                            c0, c1 = dh * 512, min((dh + 1) * 512, d)
                            nc.tensor.matmul(
                                pss[t][:, c0:c1],
                                hbm[:, t * P:(t + 1) * P],
                                w2m[:, c0:c1],
                                start=(m == 0),
                                stop=(m == mt_n - 1),
                            )
                for t in range(tt_n):
                    yt = ypool.tile([P, d], f32, tag="yt")
                    nc.vector.tensor_copy(yt[:], pss[t][:, 0:d])
                    nc.sync.dma_start(y_d[tg * tt_n + t], yt[:])


def _get_nc(d, h, ntok, debug=False, reps=1):
    key = (d, h, ntok, debug, reps)
    if key not in _NC_CACHE:
        _NC_CACHE[key] = _build_nc(d, h, ntok, debug, reps=reps)
    return _NC_CACHE[key]


# --------------------------------------------------------------------------
# Host-side input layout per core
# --------------------------------------------------------------------------

def _core_inputs(disp_e, w1_e, b1_e):
    """disp_e: [CAP, D], w1_e: [D, H], b1_e: [H] -> device input dict."""
    xt = np.ascontiguousarray(disp_e.T).reshape(D // P, P, CAP)
    w1t = np.ascontiguousarray(
        w1_e.reshape(D // P, P, H // P, P).transpose(2, 1, 0, 3))
    w2c = None  # filled by caller
    b1t = np.ascontiguousarray(b1_e.reshape(H // P, P).T)
    return xt, w1t, b1t


def _get_runner(nc, n_cores):
    """Cached PJRT executable for an SPMD bass program (axon path of
    run_bass_kernel_spmd, with the jitted callable kept warm across calls)."""
    key = id(nc)
    if key in _NC_CACHE:
        return _NC_CACHE[key]

    import jax
    from jax.sharding import Mesh, PartitionSpec
    from jax.experimental.shard_map import shard_map
    from concourse import mybir
    from concourse.bass2jax import (_bass_exec_p, install_neuronx_cc_hook,
                                    partition_id_tensor)

    install_neuronx_cc_hook()

    partition_name = (nc.partition_id_tensor.name
                      if nc.partition_id_tensor else None)
    in_names, out_names, out_avals = [], [], []
    for alloc in nc.m.functions[0].allocations:
        if not isinstance(alloc, mybir.MemoryLocationSet):
            continue
        name = alloc.memorylocations[0].name
        if alloc.kind == "ExternalInput":
            if name != partition_name:
                in_names.append(name)
        elif alloc.kind == "ExternalOutput":
            out_names.append(name)
            shape = tuple(alloc.tensor_shape)
            out_avals.append(jax.core.ShapedArray(shape, mybir.dt.np(alloc.dtype)))
    n_params = len(in_names)
    n_outs = len(out_avals)
    in_names = in_names + out_names
    if partition_name is not None:
        in_names.append(partition_name)
    donate = tuple(range(n_params, n_params + n_outs))

    def _body(*args):
        operands = list(args)
        if partition_name is not None:
            operands.append(partition_id_tensor())
        outs = _bass_exec_p.bind(
            *operands,
            out_avals=tuple(out_avals),
            in_names=tuple(in_names),
            out_names=tuple(out_names),
            lowering_input_output_aliases=(),
            sim_require_finite=True,
            sim_require_nnan=True,
            nc=nc,
        )
        return tuple(outs)

    devices = jax.devices()[:n_cores]
    mesh = Mesh(np.asarray(devices), ("core",))
    in_specs = (PartitionSpec("core"),) * (n_params + n_outs)
    out_specs = (PartitionSpec("core"),) * n_outs
    sharded = jax.jit(
        shard_map(_body, mesh=mesh, in_specs=in_specs, out_specs=out_specs,
                  check_rep=False),
        donate_argnums=donate, keep_unused=True,
    )

    def run(in_maps, reps=1, time_reps=False):
        import time as _time
        concat_in = [
            np.concatenate([np.asarray(m[in_names[i]]) for m in in_maps], axis=0)
            for i in range(n_params)
        ]
        concat_in = [jax.device_put(a) for a in concat_in]
        zero_sets = []
        for _ in range(reps):
            zero_sets.append([
                jax.device_put(np.zeros((n_cores * av.shape[0], *av.shape[1:]),
                                        av.dtype))
                for av in out_avals
            ])
        for zs in zero_sets:
            for z in zs:
                z.block_until_ready()
        for a in concat_in:
            a.block_until_ready()
        times = []
        out_arrs = None
        for r in range(reps):
            t0 = _time.perf_counter()
            out_arrs = sharded(*concat_in, *zero_sets[r])
            for o in out_arrs:
                o.block_until_ready()
            times.append(_time.perf_counter() - t0)
        results = [
            {name: np.asarray(out_arrs[i]).reshape(n_cores, *out_avals[i].shape)[c]
             for i, name in enumerate(out_names)}
            for c in range(n_cores)
        ]
        if time_reps:
            return results, times
        return results

    _NC_CACHE[key] = run
    return run


def kernel(x, wg, w1, b1, w2, b2):

    x = np.asarray(x, np.float32)
    wg = np.asarray(wg, np.float32)
    w1 = np.asarray(w1, np.float32)
    b1 = np.asarray(b1, np.float32)
    w2 = np.asarray(w2, np.float32)
    b2 = np.asarray(b2, np.float32)

    xt = x.reshape(N_TOK, D)
    gidx, gvals, pos, keep = _route(xt, wg)

    # dispatch: slots are unique per expert, so assignment == scatter-add
    disp = np.zeros((E, CAP, D), np.float32)
    for kk in range(K):
        tok = np.nonzero(keep[:, kk])[0]
        disp[gidx[tok, kk], pos[tok, kk]] = xt[tok]

    in_maps = []
    for e in range(E):
        xte, w1te, b1te = _core_inputs(disp[e], w1[e], b1[e])
        in_maps.append({
            "xt": xte,
            "w1t": w1te,
            "w2c": np.ascontiguousarray(w2[e]).reshape(H // P, P, D),
            "b1t": b1te,
        })

    nc = _get_nc(D, H, CAP)
    run = _get_runner(nc, E)
    results = run(in_maps)
    y_all = np.stack([r["y"].reshape(CAP, D) for r in results])  # [E,CAP,D]

    # combine: out = sum_k gvals * (y[e, pos] + b2[e])
    e_flat = gidx.reshape(-1)
    p_flat = pos.reshape(-1)
    yk = y_all[e_flat, p_flat] + b2[e_flat]
    w = gvals.reshape(-1).astype(np.float32)
    out = (yk * w[:, None]).reshape(N_TOK, K, D).sum(axis=1)
    return out.reshape(B, S, D).astype(np.float32)


# --------------------------------------------------------------------------
# Benchmarking helpers (test.py only)
# --------------------------------------------------------------------------

def _build_null_nc():
    """Minimal kernel (one 64KB copy) to calibrate per-call dispatch overhead."""
    from concourse import bacc, mybir, tile

    f32 = mybir.dt.float32
    nc = bacc.Bacc("TRN2", target_bir_lowering=False, debug=False,
                   enable_asserts=False, num_devices=1)
    a_d = nc.dram_tensor("a", [P, P], f32, kind="ExternalInput")
    o_d = nc.dram_tensor("o", [P, P], f32, kind="ExternalOutput")
    with tile.TileContext(nc) as tc:
        with tc.tile_pool(name="pool", bufs=1) as pool:
            t = pool.tile([P, P], f32)
            nc.sync.dma_start(t[:], a_d[:])
            nc.sync.dma_start(o_d[:], t[:])
    nc.compile()
    return nc


def bench(x, wg, w1, b1, w2, b2, reps=10):
    """Returns (real_times, null_times) lists of per-call wall seconds."""
    x = np.asarray(x, np.float32)
    xt = x.reshape(N_TOK, D)
    gidx, gvals, pos, keep = _route(xt, np.asarray(wg, np.float32))
    disp = np.zeros((E, CAP, D), np.float32)
    for kk in range(K):
        tok = np.nonzero(keep[:, kk])[0]
        disp[gidx[tok, kk], pos[tok, kk]] = xt[tok]
    in_maps = []
    for e in range(E):
        xte, w1te, b1te = _core_inputs(disp[e], np.asarray(w1[e], np.float32),
                                       np.asarray(b1[e], np.float32))
        in_maps.append({
            "xt": xte,
            "w1t": w1te,
            "w2c": np.ascontiguousarray(np.asarray(w2[e], np.float32)).reshape(H // P, P, D),
            "b1t": b1te,
        })
    nc1 = _get_nc(D, H, CAP, reps=1)
    run1 = _get_runner(nc1, E)
    _, t1 = run1(in_maps, reps=reps, time_reps=True)

    nc3 = _get_nc(D, H, CAP, reps=3)
    run3 = _get_runner(nc3, E)
    _, t3 = run3(in_maps, reps=reps, time_reps=True)
    return t1, t3


# revision 21
# speedup vs baseline: 73.4808x; 1.2343x over previous
"""MoE (GPT MLP, top-2, GShard capacity) kernel for 8 Trainium2 NeuronCores.

Strategy (expert-parallel, matching the sharding hint):
  - Host: fp32 gate (softmax + top-2 + GShard capacity positions), dispatch
    gather.  Routing is O(N*E) int/scalar work - negligible next to the FFN -
    and the capacity scan is inherently sequential, so it runs on host.
  - Device: 8 cores, core e owns expert e.  Each core runs the expert FFN
    y = gelu(disp @ w1 + b1) @ w2 over its cap=2048 dispatched token slots.
    All matmuls in fp32r (exact fp32 numerics at full PE rate).
  - Host: combine (gather + gate-weighted sum) + b2.

Self-contained: hardcodes B=4, S=2048, D=1024, H=4096, E=8, K=2, cap=2048.
"""

import sys

sys.path.insert(0, "/opt/trn_rl_repo")

import numpy as np

B, S, D, H, E = 4, 2048, 1024, 4096, 8
K = 2
N_TOK = B * S            # 8192
CAP = (K * N_TOK) // E   # 2048 (capacity factor 1.0)
EPS = 1e-9
P = 128                  # SBUF partitions

_NC_CACHE = {}


# --------------------------------------------------------------------------
# Host routing (replicates reference.py's gate exactly, in numpy fp32)
# --------------------------------------------------------------------------

def _route(xt, wg):
    """xt: [N, D] fp32, wg: [D, E] fp32 ->
    gidx [N,K] int, gvals [N,K] fp32 (keep-masked), pos [N,K] int, keep [N,K]"""
    logits = xt @ wg                                   # [N, E] fp32
    m = logits.max(axis=-1, keepdims=True)
    ex = np.exp(logits - m)
    scores = ex / ex.sum(axis=-1, keepdims=True)
    order = np.argsort(-scores, axis=1, kind="stable")  # jax top_k tie rule
    gidx = order[:, :K]                                 # [N, K]
    gvals = np.take_along_axis(scores, gidx, axis=1)
    gvals = gvals / np.clip(gvals.sum(-1, keepdims=True), EPS, None)

    n = xt.shape[0]
    offset = np.zeros(E, np.int64)
    pos = np.zeros((n, K), np.int64)
    keep = np.zeros((n, K), bool)
    rows = np.arange(n)
    for kk in range(K):
        ek = gidx[:, kk]
        oh = np.zeros((n, E), np.int64)
        oh[rows, ek] = 1
        loc = np.cumsum(oh, axis=0) - 1 + offset[None, :]
        offset = offset + oh.sum(axis=0)
        p = loc[rows, ek]
        kmask = p < CAP
        pos[:, kk] = np.where(kmask, p, 0)
        keep[:, kk] = kmask
    gvals = (gvals * keep).astype(np.float32)
    return gidx, gvals, pos, keep


# --------------------------------------------------------------------------
# Device kernel builder (one expert FFN per core, SPMD)
# --------------------------------------------------------------------------

def _build_nc(d, h, ntok, debug=False, act="Gelu", reps=1):
    """Expert FFN: y[ntok, d] = gelu(x[ntok, d] @ w1[d, h] + b1[h]) @ w2[h, d].

    Token-group-major: for each 512-token group, phase A computes all of h
    for those tokens (h stays in SBUF), then phase B contracts it against a
    fresh stream of w2.  Weights stream once per group; x and y stream once.

    Device inputs (pre-laid-out on host for contiguous DMA):
      xt  : [d/P, P, ntok]        x^T tiles (partition = D)
      w1t : [h/P, P, d/P, P]      w1 col-chunks: [m][dp][dt][j] (lhsT layout)
      w2c : [h/P, P, d]           w2 row-chunks
      b1t : [P, h/P]              b1 transposed
    Output:
      y   : [ntok/P, P, d]
    """
    from concourse import bacc, mybir, tile

    dt_n = d // P            # D tiles (contraction tiles for matmul1)
    mt_n = h // P            # H tiles
    tgs = min(512, ntok)     # tokens per group (PSUM capacity bound)
    tg_n = ntok // tgs
    tt_n = tgs // P          # 128-token tiles per group
    dh_n = (d + 511) // 512  # output D splits (PSUM bank = 512 fp32)

    f32 = mybir.dt.float32
    f32r = mybir.dt.float32r
    actf = getattr(mybir.ActivationFunctionType, act)

    nc = bacc.Bacc("TRN2", target_bir_lowering=False, debug=debug,
                   enable_asserts=False, num_devices=1)

    xt_d = nc.dram_tensor("xt", [dt_n, P, ntok], f32r, kind="ExternalInput")
    w1_d = nc.dram_tensor("w1t", [mt_n, P, dt_n, P], f32r, kind="ExternalInput")
    w2_d = nc.dram_tensor("w2c", [mt_n, P, d], f32r, kind="ExternalInput")
    b1_d = nc.dram_tensor("b1t", [P, mt_n], f32, kind="ExternalInput")
    y_d = nc.dram_tensor("y", [ntok // P, P, d], f32, kind="ExternalOutput")

    with tile.TileContext(nc) as tc:
        with (
            tc.tile_pool(name="xgpool", bufs=2 * dt_n) as xgpool,
            tc.tile_pool(name="cpool", bufs=1) as cpool,
            tc.tile_pool(name="w1pool", bufs=6) as w1pool,
            tc.tile_pool(name="hpool", bufs=mt_n + 4) as hpool,
            tc.tile_pool(name="w2pool", bufs=8) as w2pool,
            tc.tile_pool(name="ypool", bufs=4) as ypool,
            tc.tile_pool(name="psum", bufs=4, space="PSUM") as psum,
        ):
            b1_t = cpool.tile([P, mt_n], f32)
            nc.sync.dma_start(b1_t[:], b1_d[:])

            for rep in range(reps):
                for tg in range(tg_n):
                    t0, t1 = tg * tgs, (tg + 1) * tgs
                    # x slices for this token group
                    xg = []
                    for dti in range(dt_n):
                        xg_t = xgpool.tile([P, tgs], f32r, tag="xg",
                                           name=f"xg_{rep}_{tg}_{dti}")
                        nc.sync.dma_start(xg_t[:], xt_d[dti][:, t0:t1])
                        xg.append(xg_t)

                    # phase A: h tiles for this group (kept in SBUF)
                    hs = []
                    for m in range(mt_n):
                        w1m = w1pool.tile([P, dt_n, P], f32r, tag="w1m")
                        nc.sync.dma_start(w1m[:], w1_d[m])
                        ps = psum.tile([P, 512 * dh_n], f32, tag="ps")
                        for dti in range(dt_n):
                            nc.tensor.matmul(
                                ps[:, 0:tgs],
                                w1m[:, dti, :],
                                xg[dti][:],
                                start=(dti == 0),
                                stop=(dti == dt_n - 1),
                            )
                        hm = hpool.tile([P, tgs], f32r, tag="hm",
                                        name=f"hm_{rep}_{tg}_{m}")
                        nc.scalar.activation(
                            hm[:], ps[:, 0:tgs], actf,
                            bias=b1_t[:, m:m + 1], scale=1.0,
                        )
                        hs.append(hm)

                    # phase B: y = h @ w2 for this group
                    pss = [psum.tile([P, 512 * dh_n], f32, tag="ps",
                                     name=f"pss_{rep}_{tg}_{t}")
                           for t in range(tt_n)]
                    for m in range(mt_n):
                        w2m = w2pool.tile([P, d], f32r, tag="w2m")
                        nc.sync.dma_start(w2m[:], w2_d[m])
                        for t in range(tt_n):
                            for dh in range(dh_n):
                                c0, c1 = dh * 512, min((dh + 1) * 512, d)
                                nc.tensor.matmul(
                                    pss[t][:, c0:c1],
                                    hs[m][:, t * P:(t + 1) * P],
                                    w2m[:, c0:c1],
                                    start=(m == 0),
                                    stop=(m == mt_n - 1),
                                )
                    for t in range(tt_n):
                        yt = ypool.tile([P, d], f32, tag="yt")
                        nc.vector.tensor_copy(yt[:], pss[t][:, 0:d])
                        nc.sync.dma_start(y_d[tg * tt_n + t], yt[:])

    nc.compile()
    return nc


def _get_nc(d, h, ntok, debug=False, reps=1):
    key = (d, h, ntok, debug, reps)
    if key not in _NC_CACHE:
        _NC_CACHE[key] = _build_nc(d, h, ntok, debug, reps=reps)
    return _NC_CACHE[key]


# --------------------------------------------------------------------------
# Host-side input layout per core
# --------------------------------------------------------------------------

def _core_inputs(disp_e, w1_e, b1_e):
    """disp_e: [CAP, D], w1_e: [D, H], b1_e: [H] -> device input dict."""
    xt = np.ascontiguousarray(disp_e.T).reshape(D // P, P, CAP)
    w1t = np.ascontiguousarray(
        w1_e.reshape(D // P, P, H // P, P).transpose(2, 1, 0, 3))
    w2c = None  # filled by caller
    b1t = np.ascontiguousarray(b1_e.reshape(H // P, P).T)
    return xt, w1t, b1t


def _get_runner(nc, n_cores):
    """Cached PJRT executable for an SPMD bass program (axon path of
    run_bass_kernel_spmd, with the jitted callable kept warm across calls)."""
    key = id(nc)
    if key in _NC_CACHE:
        return _NC_CACHE[key]

    import jax
    from jax.sharding import Mesh, PartitionSpec
    from jax.experimental.shard_map import shard_map
    from concourse import mybir
    from concourse.bass2jax import (_bass_exec_p, install_neuronx_cc_hook,
                                    partition_id_tensor)

    install_neuronx_cc_hook()

    partition_name = (nc.partition_id_tensor.name
                      if nc.partition_id_tensor else None)
    in_names, out_names, out_avals = [], [], []
    for alloc in nc.m.functions[0].allocations:
        if not isinstance(alloc, mybir.MemoryLocationSet):
            continue
        name = alloc.memorylocations[0].name
        if alloc.kind == "ExternalInput":
            if name != partition_name:
                in_names.append(name)
        elif alloc.kind == "ExternalOutput":
            out_names.append(name)
            shape = tuple(alloc.tensor_shape)
            out_avals.append(jax.core.ShapedArray(shape, mybir.dt.np(alloc.dtype)))
    n_params = len(in_names)
    n_outs = len(out_avals)
    in_names = in_names + out_names
    if partition_name is not None:
        in_names.append(partition_name)
    donate = tuple(range(n_params, n_params + n_outs))

    def _body(*args):
        operands = list(args)
        if partition_name is not None:
            operands.append(partition_id_tensor())
        outs = _bass_exec_p.bind(
            *operands,
            out_avals=tuple(out_avals),
            in_names=tuple(in_names),
            out_names=tuple(out_names),
            lowering_input_output_aliases=(),
            sim_require_finite=True,
            sim_require_nnan=True,
            nc=nc,
        )
        return tuple(outs)

    devices = jax.devices()[:n_cores]
    mesh = Mesh(np.asarray(devices), ("core",))
    in_specs = (PartitionSpec("core"),) * (n_params + n_outs)
    out_specs = (PartitionSpec("core"),) * n_outs
    sharded = jax.jit(
        shard_map(_body, mesh=mesh, in_specs=in_specs, out_specs=out_specs,
                  check_rep=False),
        donate_argnums=donate, keep_unused=True,
    )

    def run(in_maps, reps=1, time_reps=False):
        import time as _time
        concat_in = [
            np.concatenate([np.asarray(m[in_names[i]]) for m in in_maps], axis=0)
            for i in range(n_params)
        ]
        concat_in = [jax.device_put(a) for a in concat_in]
        zero_sets = []
        for _ in range(reps):
            zero_sets.append([
                jax.device_put(np.zeros((n_cores * av.shape[0], *av.shape[1:]),
                                        av.dtype))
                for av in out_avals
            ])
        for zs in zero_sets:
            for z in zs:
                z.block_until_ready()
        for a in concat_in:
            a.block_until_ready()
        times = []
        out_arrs = None
        for r in range(reps):
            t0 = _time.perf_counter()
            out_arrs = sharded(*concat_in, *zero_sets[r])
            for o in out_arrs:
                o.block_until_ready()
            times.append(_time.perf_counter() - t0)
        results = [
            {name: np.asarray(out_arrs[i]).reshape(n_cores, *out_avals[i].shape)[c]
             for i, name in enumerate(out_names)}
            for c in range(n_cores)
        ]
        if time_reps:
            return results, times
        return results

    _NC_CACHE[key] = run
    return run


def kernel(x, wg, w1, b1, w2, b2):

    x = np.asarray(x, np.float32)
    wg = np.asarray(wg, np.float32)
    w1 = np.asarray(w1, np.float32)
    b1 = np.asarray(b1, np.float32)
    w2 = np.asarray(w2, np.float32)
    b2 = np.asarray(b2, np.float32)

    xt = x.reshape(N_TOK, D)
    gidx, gvals, pos, keep = _route(xt, wg)

    # dispatch: slots are unique per expert, so assignment == scatter-add
    disp = np.zeros((E, CAP, D), np.float32)
    for kk in range(K):
        tok = np.nonzero(keep[:, kk])[0]
        disp[gidx[tok, kk], pos[tok, kk]] = xt[tok]

    in_maps = []
    for e in range(E):
        xte, w1te, b1te = _core_inputs(disp[e], w1[e], b1[e])
        in_maps.append({
            "xt": xte,
            "w1t": w1te,
            "w2c": np.ascontiguousarray(w2[e]).reshape(H // P, P, D),
            "b1t": b1te,
        })

    nc = _get_nc(D, H, CAP)
    run = _get_runner(nc, E)
    results = run(in_maps)
    y_all = np.stack([r["y"].reshape(CAP, D) for r in results])  # [E,CAP,D]

    # combine: out = sum_k gvals * (y[e, pos] + b2[e])
    e_flat = gidx.reshape(-1)
    p_flat = pos.reshape(-1)
    yk = y_all[e_flat, p_flat] + b2[e_flat]
    w = gvals.reshape(-1).astype(np.float32)
    out = (yk * w[:, None]).reshape(N_TOK, K, D).sum(axis=1)
    return out.reshape(B, S, D).astype(np.float32)


# --------------------------------------------------------------------------
# Benchmarking helpers (test.py only)
# --------------------------------------------------------------------------

def _build_null_nc():
    """Minimal kernel (one 64KB copy) to calibrate per-call dispatch overhead."""
    from concourse import bacc, mybir, tile

    f32 = mybir.dt.float32
    nc = bacc.Bacc("TRN2", target_bir_lowering=False, debug=False,
                   enable_asserts=False, num_devices=1)
    a_d = nc.dram_tensor("a", [P, P], f32, kind="ExternalInput")
    o_d = nc.dram_tensor("o", [P, P], f32, kind="ExternalOutput")
    with tile.TileContext(nc) as tc:
        with tc.tile_pool(name="pool", bufs=1) as pool:
            t = pool.tile([P, P], f32)
            nc.sync.dma_start(t[:], a_d[:])
            nc.sync.dma_start(o_d[:], t[:])
    nc.compile()
    return nc


def bench(x, wg, w1, b1, w2, b2, reps=10):
    """Returns (real_times, null_times) lists of per-call wall seconds."""
    x = np.asarray(x, np.float32)
    xt = x.reshape(N_TOK, D)
    gidx, gvals, pos, keep = _route(xt, np.asarray(wg, np.float32))
    disp = np.zeros((E, CAP, D), np.float32)
    for kk in range(K):
        tok = np.nonzero(keep[:, kk])[0]
        disp[gidx[tok, kk], pos[tok, kk]] = xt[tok]
    in_maps = []
    for e in range(E):
        xte, w1te, b1te = _core_inputs(disp[e], np.asarray(w1[e], np.float32),
                                       np.asarray(b1[e], np.float32))
        in_maps.append({
            "xt": xte,
            "w1t": w1te,
            "w2c": np.ascontiguousarray(np.asarray(w2[e], np.float32)).reshape(H // P, P, D),
            "b1t": b1te,
        })
    nc1 = _get_nc(D, H, CAP, reps=1)
    run1 = _get_runner(nc1, E)
    _, t1 = run1(in_maps, reps=reps, time_reps=True)

    nc5 = _get_nc(D, H, CAP, reps=5)
    run5 = _get_runner(nc5, E)
    _, t5 = run5(in_maps, reps=reps, time_reps=True)
    return t1, t5
